# revision 15
# baseline (speedup 1.0000x reference)
"""Trainium2 Bass kernel for the attention-LSTM captioning RNN.

Problem (per full batch): x(64,128,512), A(64,1024,4,4), Wx(512,4096),
Wh(1024,4096), Wattn(1024,4096), b(4096) -> h-sequence (64,128,1024).

Strategy: data-parallel over N across 8 cores (8 samples/core, weights
replicated).  Per core:
  - precompute P[(n,l),g] = Af[n,:,l] @ Wattn  (PE, fp32) -> SBUF bf16
  - precompute xWx^T (gate-major) once (PE, f32r) -> SBUF bf16, indexed
    per step with a strided AP (no per-step DMA)
  - recurrence in transposed ("a^T") layout: gates live on 128 partitions
    (partition = gate-col % 128), batch (8) on the free dim.  Wh is the
    stationary operand (bf16, FWL), h^T the 8-wide moving operand.
    Attention is folded in as a second accumulating matmul with a
    block-diagonal softmax-weight matrix E (128x8) against stationary P.
Host numpy does all layout transposes (free: not timed on device).
"""

import math
import sys

sys.path.insert(0, "/root/shim")
sys.path.insert(0, "/opt/trn_rl_repo")

import numpy as np
import ml_dtypes

try:
    import antenv

    if "/root/shim/antenv" not in list(antenv.__path__):
        antenv.__path__.append("/root/shim/antenv")
except Exception:
    pass

import concourse.bass as bass
import concourse.bacc as bacc
import concourse.mybir as mybir
from concourse.tile import TileContext
from concourse.bass_utils import run_bass_kernel_spmd

FP32 = mybir.dt.float32
F32R = mybir.dt.float32r
BF16 = mybir.dt.bfloat16

# Problem constants (hardcoded per harness contract)
N, T, D, H = 64, 128, 512, 1024
NC = 8            # cores
NL = N // NC      # samples per core = 8
G = 4 * H         # 4096 gate columns
L = 16            # attention locations
HC = H // 128     # 8 h-chunks
GM = G // 128     # 32 gate-col chunks
DC = D // 128     # 4 d-chunks
INV_SQRT_H = 1.0 / math.sqrt(H)


def build_nc(timesteps=T):
    nc = bacc.Bacc()

    # ---- DRAM I/O (host-prepped layouts) ----
    xT_d = nc.dram_tensor("xT", [128, DC, NL, timesteps], FP32, kind="ExternalInput")
    afT_d = nc.dram_tensor("afT", [128, HC, NL, L], FP32, kind="ExternalInput")
    wx_d = nc.dram_tensor("wx", [128, DC, G], FP32, kind="ExternalInput")
    wh_d = nc.dram_tensor("wh", [128, HC, G], BF16, kind="ExternalInput")
    wattn_d = nc.dram_tensor("wattn", [128, HC, G], FP32, kind="ExternalInput")
    b_d = nc.dram_tensor("bias", [128, GM], FP32, kind="ExternalInput")
    mask_d = nc.dram_tensor("mask", [128, NL], FP32, kind="ExternalInput")
    out_d = nc.dram_tensor("hsT", [timesteps, HC, NL, 128], FP32, kind="ExternalOutput")
    dbg_d = nc.dram_tensor("dbg", [timesteps, GM, NL, 128], FP32, kind="ExternalOutput")
    dbg2_d = nc.dram_tensor("dbg2", [timesteps, HC, NL, 128], BF16, kind="ExternalOutput")

    with TileContext(nc) as tc:
        # ---------- persistent SBUF ----------
        with tc.tile_pool(name="persist", bufs=1) as pp:
            afTb = pp.tile([128, HC, NL, L], BF16)     # Af^T bf16, (hc,n,l) free
            p_sb = pp.tile([128, G], BF16)             # P[(n,l), g]
            wh_sb = pp.tile([128, HC, G], BF16)        # Wh tiles
            xwxt = pp.tile([128, GM, NL, timesteps], BF16)  # xWx^T (+bias)
            bias_sb = pp.tile([128, GM], FP32)
            mask_sb = pp.tile([128, NL], FP32)
            ones_sb = pp.tile([128, 1], FP32)          # for partition-sum matmul
            one1_sb = pp.tile([1, 1], FP32)            # identity for 1xF transpose
            hT32 = pp.tile([128, HC, NL], FP32)        # h^T fp32 (output + c path)
            hTb = pp.tile([128, HC, NL], BF16)         # h^T bf16 (matmul operand)
            cT = pp.tile([128, HC, NL], FP32)

            nc.sync.dma_start(bias_sb[:], b_d[:])
            nc.sync.dma_start(mask_sb[:], mask_d[:])
            nc.vector.memset(ones_sb[:], 1.0)
            nc.vector.memset(one1_sb[:], 1.0)

            # ---------- P = Af^T @ Wattn  (fp32, one-time) ----------
            with (
                tc.tile_pool(name="wattn", bufs=1) as wap,
                tc.tile_pool(name="wsl", bufs=2) as wslp,
                tc.tile_pool(name="ppsum", bufs=1, space="PSUM") as ppp,
            ):
                afT = wap.tile([128, HC, NL, L], FP32)
                nc.sync.dma_start(afT[:], afT_d[:])
                nc.vector.tensor_copy(afTb[:], afT[:])

                # h0 = mean over l of Af  (reduce innermost l)
                nc.vector.tensor_reduce(
                    hT32[:], afT[:], axis=mybir.AxisListType.X,
                    op=mybir.AluOpType.add,
                )
                nc.vector.tensor_scalar_mul(hT32[:], hT32[:], 1.0 / L)
                nc.vector.tensor_copy(cT[:], hT32[:])
                nc.vector.tensor_copy(hTb[:], hT32[:])

                pps = [ppp.tile([128, 1024], FP32, tag=f"pps{gc}", name=f"pps{gc}") for gc in range(4)]
                for hc in range(HC):
                    wsl = wslp.tile([128, G], FP32, tag="wsl")
                    nc.sync.dma_start(wsl[:], wattn_d[:, hc, :])
                    for gc in range(4):
                        for hf in range(2):
                            nc.tensor.matmul(
                                pps[gc][:, hf * 512:(hf + 1) * 512],
                                afT[:, hc, :, :],
                                wsl[
                                    :,
                                    gc * 1024 + hf * 512:gc * 1024 + (hf + 1) * 512,
                                ],
                                start=(hc == 0),
                                stop=(hc == HC - 1),
                            )
                for gc in range(4):
                    nc.vector.tensor_copy(
                        p_sb[:, gc * 1024:(gc + 1) * 1024], pps[gc][:]
                    )

            # ---------- xWx^T into SBUF bf16 (f32r, one-time) ----------
            with (
                tc.tile_pool(name="xwx", bufs=1) as xp,
                tc.tile_pool(name="xwxs", bufs=2) as xsp,
                tc.tile_pool(name="xwpsum", bufs=1, space="PSUM") as xpp,
            ):
                xT_r = xp.tile([128, DC, NL, timesteps], F32R)
                for dc in range(DC):
                    st2 = xsp.tile([128, NL * timesteps], FP32, tag="stage2")
                    nc.sync.dma_start(
                        st2[:], xT_d[:, dc, :, :].rearrange("p n t -> p (n t)")
                    )
                    nc.vector.tensor_copy(
                        xT_r[:, dc, :, :].rearrange("p n t -> p (n t)"), st2[:]
                    )
                ncols = NL * timesteps  # 1024
                col_chunks = [(s, min(s + 512, ncols)) for s in range(0, ncols, 512)]
                for mg in range(GM // 4):  # groups of 4 gate-chunks
                    xwg = [
                        xpp.tile([128, ncols], FP32, tag=f"xw{i}", name=f"xw{i}")
                        for i in range(4)
                    ]
                    for dc in range(DC):
                        st = xsp.tile([128, 512], FP32, tag="stage")
                        nc.sync.dma_start(
                            st[:], wx_d[:, dc, mg * 512:(mg + 1) * 512]
                        )
                        wxr = xsp.tile([128, 512], F32R, tag="wxr")
                        nc.vector.tensor_copy(wxr[:], st[:])
                        for i in range(4):
                            for (lo, hi) in col_chunks:
                                nc.tensor.matmul(
                                    xwg[i][:, lo:hi],
                                    wxr[:, i * 128:(i + 1) * 128],
                                    xT_r[:, dc, :, :].rearrange(
                                        "p n t -> p (n t)"
                                    )[:, lo:hi],
                                    start=(dc == 0),
                                    stop=(dc == DC - 1),
                                )
                    for i in range(4):
                        m = mg * 4 + i
                        nc.vector.tensor_scalar_add(
                            xwxt[:, m, :, :].rearrange("p n t -> p (n t)"),
                            xwg[i][:],
                            bias_sb[:, m:m + 1],
                        )

            # Wh load (bf16, direct)
            nc.sync.dma_start(wh_sb[:], wh_d[:])

            # ---------- recurrence ----------
            with (
                tc.tile_pool(name="step", bufs=2) as sp,
                tc.tile_pool(name="gpsum", bufs=2, space="PSUM") as gp,
                tc.tile_pool(name="spsum", bufs=2, space="PSUM") as ssp,
            ):
                with tc.For_i(0, timesteps, 1) as ti:
                    nc.sync.dma_start(
                        dbg2_d[bass.ds(ti, 1), :, :, :].rearrange(
                            "t c n p -> p (t c) n"
                        ),
                        hTb[:],
                    )
                    aT = gp.tile([128, GM, NL], FP32, tag="aT")
                    # gates = Wh^T-tiles @ h^T   (256 bf16 matmuls)
                    for m in range(GM):
                        for kc in range(HC):
                            nc.tensor.matmul(
                                aT[:, m, :],
                                wh_sb[:, kc, m * 128:(m + 1) * 128],
                                hTb[:, kc, :],
                                start=(kc == 0),
                                stop=(kc == HC - 1),
                            )

                    # ----- attention scores from h (pre-update) -----
                    z = sp.tile([128, NL, L, HC], FP32, tag="z")
                    nc.vector.tensor_tensor(
                        z[:],
                        afTb[:].transpose([0, 2, 3, 1]),   # (p, n, l, hc)
                        hTb[:].transpose([0, 2, 1]).unsqueeze(2).broadcast_to(
                            [128, NL, L, HC]
                        ),
                        mybir.AluOpType.mult,
                    )
                    z2 = sp.tile([128, NL * L], FP32, tag="z2")
                    nc.vector.tensor_reduce(
                        z2[:], z[:], axis=mybir.AxisListType.X, op=mybir.AluOpType.add
                    )
                    sc = ssp.tile([1, NL * L], FP32, tag="sc")
                    nc.tensor.matmul(sc[:], ones_sb[:], z2[:], start=True, stop=True)
                    mx = sp.tile([1, NL], FP32, tag="mx")
                    nc.vector.tensor_reduce(
                        mx[:],
                        sc[:].rearrange("q (n l) -> q n l", n=NL),
                        axis=mybir.AxisListType.X,
                        op=mybir.AluOpType.max,
                    )
                    sd = sp.tile([1, NL * L], FP32, tag="sd")
                    nc.vector.tensor_tensor(
                        sd[:].rearrange("q (n l) -> q n l", n=NL),
                        sc[:].rearrange("q (n l) -> q n l", n=NL),
                        mx[:].unsqueeze(2).broadcast_to([1, NL, L]),
                        mybir.AluOpType.subtract,
                    )
                    ex = sp.tile([1, NL * L], FP32, tag="ex")
                    nc.scalar.activation(
                        ex[:], sd[:], mybir.ActivationFunctionType.Exp,
                        scale=INV_SQRT_H,
                    )
                    zs = sp.tile([1, NL], FP32, tag="zs")
                    nc.vector.tensor_reduce(
                        zs[:],
                        ex[:].rearrange("q (n l) -> q n l", n=NL),
                        axis=mybir.AxisListType.X,
                        op=mybir.AluOpType.add,
                    )
                    rz = sp.tile([1, NL], FP32, tag="rz")
                    nc.vector.reciprocal(rz[:], zs[:])
                    w = sp.tile([1, NL * L], FP32, tag="w")
                    nc.vector.tensor_tensor(
                        w[:].rearrange("q (n l) -> q n l", n=NL),
                        ex[:].rearrange("q (n l) -> q n l", n=NL),
                        rz[:].unsqueeze(2).broadcast_to([1, NL, L]),
                        mybir.AluOpType.mult,
                    )
                    wT = ssp.tile([128, 1], FP32, tag="wT")
                    nc.tensor.transpose(wT[:], w[:], one1_sb[:])
                    ee = sp.tile([128, NL], BF16, tag="ee")
                    nc.vector.tensor_tensor(
                        ee[:],
                        mask_sb[:],
                        wT[:].broadcast_to([128, NL]),
                        mybir.AluOpType.mult,
                    )

                    # attention contribution in its own PSUM tile
                    uT = gp.tile([128, GM, NL], FP32, tag="uT")
                    for m in range(GM):
                        nc.tensor.matmul(
                            uT[:, m, :],
                            p_sb[:, m * 128:(m + 1) * 128],
                            ee[:],
                            start=True,
                            stop=True,
                        )

                    # ----- gate math -----
                    spre = sp.tile([128, GM, NL], FP32, tag="spre")
                    nc.vector.tensor_tensor(
                        spre[:].rearrange("p m n -> p (m n)").unsqueeze(2),
                        aT[:].rearrange("p m n -> p (m n)").unsqueeze(2),
                        xwxt[:, :, :, bass.ds(ti, 1)].rearrange("p m n t -> p (m n) t"),
                        mybir.AluOpType.add,
                    )
                    nc.vector.tensor_tensor(
                        spre[:], spre[:], uT[:], mybir.AluOpType.add
                    )
                    gs = sp.tile([128, GM, NL], FP32, tag="gs")
                    fl = spre[:].rearrange("p m n -> p (m n)")
                    gl = gs[:].rearrange("p m n -> p (m n)")
                    q = HC * NL  # 64 columns per gate
                    nc.scalar.activation(
                        gl[:, 0 * q:1 * q], fl[:, 0 * q:1 * q],
                        mybir.ActivationFunctionType.Sigmoid,
                    )
                    nc.scalar.activation(
                        gl[:, 1 * q:2 * q], fl[:, 1 * q:2 * q],
                        mybir.ActivationFunctionType.Sigmoid,
                    )
                    nc.scalar.activation(
                        gl[:, 2 * q:3 * q], fl[:, 2 * q:3 * q],
                        mybir.ActivationFunctionType.Sigmoid,
                    )
                    nc.scalar.activation(
                        gl[:, 3 * q:4 * q], fl[:, 3 * q:4 * q],
                        mybir.ActivationFunctionType.Tanh,
                    )
                    ig = sp.tile([128, HC * NL], FP32, tag="ig")
                    nc.vector.tensor_tensor(
                        ig[:], gl[:, 0 * q:1 * q], gl[:, 3 * q:4 * q],
                        mybir.AluOpType.mult,
                    )
                    cflat = cT[:].rearrange("p c n -> p (c n)")
                    nc.vector.tensor_tensor(
                        cflat, cflat, gl[:, 1 * q:2 * q], mybir.AluOpType.mult
                    )
                    nc.vector.tensor_tensor(cflat, cflat, ig[:], mybir.AluOpType.add)
                    tc_t = sp.tile([128, HC * NL], FP32, tag="tct")
                    nc.scalar.activation(
                        tc_t[:], cflat, mybir.ActivationFunctionType.Tanh
                    )
                    hflat = hT32[:].rearrange("p c n -> p (c n)")
                    nc.vector.tensor_tensor(
                        hflat, gl[:, 2 * q:3 * q], tc_t[:], mybir.AluOpType.mult
                    )
                    nc.vector.tensor_copy(hTb[:], hT32[:])
                    nc.sync.dma_start(
                        dbg_d[bass.ds(ti, 1), :, :, :].rearrange(
                            "t m n p -> p (t m) n"
                        ),
                        spre[:],
                    )
                    nc.sync.dma_start(
                        out_d[bass.ds(ti, 1), :, :, :].rearrange(
                            "t c n p -> p (t c) n"
                        ),
                        hT32[:],
                    )

    nc.finalize()
    return nc


def prep_inputs(x, A, Wx, Wh, Wattn, b):
    """Host-side reshapes to device layouts; returns per-core input maps."""
    x = np.asarray(x, dtype=np.float32)
    A = np.asarray(A, dtype=np.float32)
    Wx = np.asarray(Wx, dtype=np.float32)
    Wh = np.asarray(Wh, dtype=np.float32)
    Wattn = np.asarray(Wattn, dtype=np.float32)
    b = np.asarray(b, dtype=np.float32)
    timesteps = x.shape[1]

    # weight layouts [p, kc, g] with k = kc*128 + p
    wx_h = np.ascontiguousarray(Wx.reshape(DC, 128, G).transpose(1, 0, 2))
    wh_h = np.ascontiguousarray(
        Wh.reshape(HC, 128, G).transpose(1, 0, 2).astype(ml_dtypes.bfloat16)
    )
    wattn_h = np.ascontiguousarray(Wattn.reshape(HC, 128, G).transpose(1, 0, 2))
    b_h = np.ascontiguousarray(b.reshape(GM, 128).T)  # [p, m]
    mask_h = np.zeros((128, NL), dtype=np.float32)
    for p in range(128):
        mask_h[p, p // L] = 1.0

    in_maps = []
    for c in range(NC):
        xs = x[c * NL:(c + 1) * NL]          # (8, T, 512)
        As = A[c * NL:(c + 1) * NL].reshape(NL, H, L)  # (8, 1024, 16)
        # xT [p, dc, n, t] = x[n, t, dc*128+p]
        xT_h = np.ascontiguousarray(
            xs.reshape(NL, timesteps, DC, 128).transpose(3, 2, 0, 1)
        )
        # afT [p, hc, n, l] = Af[n, hc*128+p, l]
        afT_h = np.ascontiguousarray(
            As.reshape(NL, HC, 128, L).transpose(2, 1, 0, 3)
        )
        in_maps.append(
            {
                "xT": xT_h,
                "afT": afT_h,
                "wx": wx_h,
                "wh": wh_h,
                "wattn": wattn_h,
                "bias": b_h,
                "mask": mask_h,
            }
        )
    return in_maps


_NC_CACHE = {}


def kernel(x, A, Wx, Wh, Wattn, b, trace=False):
    timesteps = x.shape[1]
    key = timesteps
    if key not in _NC_CACHE:
        _NC_CACHE[key] = build_nc(timesteps)
    nc = _NC_CACHE[key]
    in_maps = prep_inputs(x, A, Wx, Wh, Wattn, b)
    res = run_bass_kernel_spmd(nc, in_maps, list(range(NC)), trace=trace)
    outs = []
    for c in range(NC):
        hsT = res.results[c]["hsT"]  # (T, HC, NL, 128)
        # out[n, t, hc*128+p] = hsT[t, hc, n, p]
        outs.append(hsT.transpose(2, 0, 1, 3).reshape(NL, timesteps, H))
    full = np.concatenate(outs, axis=0).astype(np.float32)
    kernel.last_result = res
    return full


# revision 16
# speedup vs baseline: 7.5258x; 7.5258x over previous
"""Trainium2 Bass kernel for the attention-LSTM captioning RNN.

Problem (per full batch): x(64,128,512), A(64,1024,4,4), Wx(512,4096),
Wh(1024,4096), Wattn(1024,4096), b(4096) -> h-sequence (64,128,1024).

Strategy: data-parallel over N across 8 cores (8 samples/core, weights
replicated).  Per core:
  - precompute P[(n,l),g] = Af[n,:,l] @ Wattn  (PE, fp32) -> SBUF bf16
  - precompute xWx^T (gate-major) once (PE, f32r) -> SBUF bf16, indexed
    per step with a strided AP (no per-step DMA)
  - recurrence in transposed ("a^T") layout: gates live on 128 partitions
    (partition = gate-col % 128), batch (8) on the free dim.  Wh is the
    stationary operand (bf16, FWL), h^T the 8-wide moving operand.
    Attention is folded in as a second accumulating matmul with a
    block-diagonal softmax-weight matrix E (128x8) against stationary P.
Host numpy does all layout transposes (free: not timed on device).
"""

import math
import sys

sys.path.insert(0, "/root/shim")
sys.path.insert(0, "/opt/trn_rl_repo")

import numpy as np
import ml_dtypes

try:
    import antenv

    if "/root/shim/antenv" not in list(antenv.__path__):
        antenv.__path__.append("/root/shim/antenv")
except Exception:
    pass

import concourse.bass as bass
import concourse.bacc as bacc
import concourse.mybir as mybir
from concourse.tile import TileContext
from concourse.bass_utils import run_bass_kernel_spmd

FP32 = mybir.dt.float32
F32R = mybir.dt.float32r
BF16 = mybir.dt.bfloat16

# Problem constants (hardcoded per harness contract)
N, T, D, H = 64, 128, 512, 1024
NC = 8            # cores
NL = N // NC      # samples per core = 8
G = 4 * H         # 4096 gate columns
L = 16            # attention locations
HC = H // 128     # 8 h-chunks
GM = G // 128     # 32 gate-col chunks
DC = D // 128     # 4 d-chunks
INV_SQRT_H = 1.0 / math.sqrt(H)


def build_nc(timesteps=T):
    nc = bacc.Bacc()

    # ---- DRAM I/O (host-prepped layouts) ----
    xT_d = nc.dram_tensor("xT", [128, DC, NL, timesteps], FP32, kind="ExternalInput")
    afT_d = nc.dram_tensor("afT", [128, HC, NL, L], FP32, kind="ExternalInput")
    wx_d = nc.dram_tensor("wx", [128, DC, G], FP32, kind="ExternalInput")
    wh_d = nc.dram_tensor("wh", [128, HC, G], BF16, kind="ExternalInput")
    wattn_d = nc.dram_tensor("wattn", [128, HC, G], FP32, kind="ExternalInput")
    b_d = nc.dram_tensor("bias", [128, GM], FP32, kind="ExternalInput")
    mask_d = nc.dram_tensor("mask", [128, NL], FP32, kind="ExternalInput")
    out_d = nc.dram_tensor("hsT", [timesteps, 128, HC, NL], FP32, kind="ExternalOutput")

    with TileContext(nc) as tc:
        # ---------- persistent SBUF ----------
        with tc.tile_pool(name="persist", bufs=1) as pp:
            afTb = pp.tile([128, HC, NL, L], BF16)     # Af^T bf16, (hc,n,l) free
            p_sb = pp.tile([128, G], BF16)             # P[(n,l), g]
            wh_sb = pp.tile([128, HC, G], BF16)        # Wh tiles
            xwxt = pp.tile([128, GM, NL, timesteps], BF16)  # xWx^T (+bias)
            bias_sb = pp.tile([128, GM], FP32)
            mask_sb = pp.tile([128, NL], FP32)
            ones_sb = pp.tile([128, 1], FP32)          # for partition-sum matmul
            one1_sb = pp.tile([1, 1], FP32)            # identity for 1xF transpose
            hT32 = pp.tile([128, HC, NL], FP32)        # h^T fp32 (output + c path)
            hTb = pp.tile([128, HC, NL], BF16)         # h^T bf16 (matmul operand)
            cT = pp.tile([128, HC, NL], FP32)

            nc.sync.dma_start(bias_sb[:], b_d[:])
            nc.sync.dma_start(mask_sb[:], mask_d[:])
            nc.vector.memset(ones_sb[:], 1.0)
            nc.vector.memset(one1_sb[:], 1.0)

            # ---------- P = Af^T @ Wattn  (fp32, one-time) ----------
            with (
                tc.tile_pool(name="wattn", bufs=1) as wap,
                tc.tile_pool(name="wsl", bufs=2) as wslp,
                tc.tile_pool(name="ppsum", bufs=1, space="PSUM") as ppp,
            ):
                afT = wap.tile([128, HC, NL, L], FP32)
                nc.sync.dma_start(afT[:], afT_d[:])
                nc.vector.tensor_copy(afTb[:], afT[:])

                # h0 = mean over l of Af  (reduce innermost l)
                nc.vector.tensor_reduce(
                    hT32[:], afT[:], axis=mybir.AxisListType.X,
                    op=mybir.AluOpType.add,
                )
                nc.vector.tensor_scalar_mul(hT32[:], hT32[:], 1.0 / L)
                nc.vector.tensor_copy(cT[:], hT32[:])
                nc.vector.tensor_copy(hTb[:], hT32[:])

                pps = [ppp.tile([128, 1024], FP32, tag=f"pps{gc}", name=f"pps{gc}") for gc in range(4)]
                for hc in range(HC):
                    wsl = wslp.tile([128, G], FP32, tag="wsl")
                    nc.sync.dma_start(wsl[:], wattn_d[:, hc, :])
                    for gc in range(4):
                        for hf in range(2):
                            nc.tensor.matmul(
                                pps[gc][:, hf * 512:(hf + 1) * 512],
                                afT[:, hc, :, :],
                                wsl[
                                    :,
                                    gc * 1024 + hf * 512:gc * 1024 + (hf + 1) * 512,
                                ],
                                start=(hc == 0),
                                stop=(hc == HC - 1),
                            )
                for gc in range(4):
                    nc.vector.tensor_copy(
                        p_sb[:, gc * 1024:(gc + 1) * 1024], pps[gc][:]
                    )

            # ---------- xWx^T into SBUF bf16 (f32r, one-time) ----------
            with (
                tc.tile_pool(name="xwx", bufs=1) as xp,
                tc.tile_pool(name="xwxs", bufs=2) as xsp,
                tc.tile_pool(name="xwpsum", bufs=1, space="PSUM") as xpp,
            ):
                xT_r = xp.tile([128, DC, NL, timesteps], F32R)
                for dc in range(DC):
                    st2 = xsp.tile([128, NL * timesteps], FP32, tag="stage2")
                    nc.sync.dma_start(
                        st2[:], xT_d[:, dc, :, :].rearrange("p n t -> p (n t)")
                    )
                    nc.vector.tensor_copy(
                        xT_r[:, dc, :, :].rearrange("p n t -> p (n t)"), st2[:]
                    )
                ncols = NL * timesteps  # 1024
                col_chunks = [(s, min(s + 512, ncols)) for s in range(0, ncols, 512)]
                for mg in range(GM // 4):  # groups of 4 gate-chunks
                    xwg = [
                        xpp.tile([128, ncols], FP32, tag=f"xw{i}", name=f"xw{i}")
                        for i in range(4)
                    ]
                    for dc in range(DC):
                        st = xsp.tile([128, 512], FP32, tag="stage")
                        nc.sync.dma_start(
                            st[:], wx_d[:, dc, mg * 512:(mg + 1) * 512]
                        )
                        wxr = xsp.tile([128, 512], F32R, tag="wxr")
                        nc.vector.tensor_copy(wxr[:], st[:])
                        for i in range(4):
                            for (lo, hi) in col_chunks:
                                nc.tensor.matmul(
                                    xwg[i][:, lo:hi],
                                    wxr[:, i * 128:(i + 1) * 128],
                                    xT_r[:, dc, :, :].rearrange(
                                        "p n t -> p (n t)"
                                    )[:, lo:hi],
                                    start=(dc == 0),
                                    stop=(dc == DC - 1),
                                )
                    for i in range(4):
                        m = mg * 4 + i
                        nc.vector.tensor_scalar_add(
                            xwxt[:, m, :, :].rearrange("p n t -> p (n t)"),
                            xwg[i][:],
                            bias_sb[:, m:m + 1],
                        )

            # Wh load (bf16, direct)
            nc.sync.dma_start(wh_sb[:], wh_d[:])

            # ---------- recurrence ----------
            with (
                tc.tile_pool(name="step", bufs=2) as sp,
                tc.tile_pool(name="gpsum", bufs=2, space="PSUM") as gp,
                tc.tile_pool(name="spsum", bufs=2, space="PSUM") as ssp,
            ):
                with tc.For_i(0, timesteps, 1) as ti:
                    aT = gp.tile([128, GM, NL], FP32, tag="aT")
                    # gates = Wh^T-tiles @ h^T   (256 bf16 matmuls)
                    for m in range(GM):
                        for kc in range(HC):
                            nc.tensor.matmul(
                                aT[:, m, :],
                                wh_sb[:, kc, m * 128:(m + 1) * 128],
                                hTb[:, kc, :],
                                start=(kc == 0),
                                stop=(kc == HC - 1),
                            )

                    # ----- attention scores from h (pre-update) -----
                    z = sp.tile([128, NL, L, HC], FP32, tag="z")
                    nc.vector.tensor_tensor(
                        z[:],
                        afTb[:].transpose([0, 2, 3, 1]),   # (p, n, l, hc)
                        hTb[:].transpose([0, 2, 1]).unsqueeze(2).broadcast_to(
                            [128, NL, L, HC]
                        ),
                        mybir.AluOpType.mult,
                    )
                    z2 = sp.tile([128, NL * L], FP32, tag="z2")
                    nc.vector.tensor_reduce(
                        z2[:], z[:], axis=mybir.AxisListType.X, op=mybir.AluOpType.add
                    )
                    sc = ssp.tile([1, NL * L], FP32, tag="sc")
                    nc.tensor.matmul(sc[:], ones_sb[:], z2[:], start=True, stop=True)
                    mx = sp.tile([1, NL], FP32, tag="mx")
                    nc.vector.tensor_reduce(
                        mx[:],
                        sc[:].rearrange("q (n l) -> q n l", n=NL),
                        axis=mybir.AxisListType.X,
                        op=mybir.AluOpType.max,
                    )
                    sd = sp.tile([1, NL * L], FP32, tag="sd")
                    nc.vector.tensor_tensor(
                        sd[:].rearrange("q (n l) -> q n l", n=NL),
                        sc[:].rearrange("q (n l) -> q n l", n=NL),
                        mx[:].unsqueeze(2).broadcast_to([1, NL, L]),
                        mybir.AluOpType.subtract,
                    )
                    ex = sp.tile([1, NL * L], FP32, tag="ex")
                    nc.scalar.activation(
                        ex[:], sd[:], mybir.ActivationFunctionType.Exp,
                        scale=INV_SQRT_H,
                    )
                    zs = sp.tile([1, NL], FP32, tag="zs")
                    nc.vector.tensor_reduce(
                        zs[:],
                        ex[:].rearrange("q (n l) -> q n l", n=NL),
                        axis=mybir.AxisListType.X,
                        op=mybir.AluOpType.add,
                    )
                    rz = sp.tile([1, NL], FP32, tag="rz")
                    nc.vector.reciprocal(rz[:], zs[:])
                    w = sp.tile([1, NL * L], FP32, tag="w")
                    nc.vector.tensor_tensor(
                        w[:].rearrange("q (n l) -> q n l", n=NL),
                        ex[:].rearrange("q (n l) -> q n l", n=NL),
                        rz[:].unsqueeze(2).broadcast_to([1, NL, L]),
                        mybir.AluOpType.mult,
                    )
                    wT = ssp.tile([128, 1], FP32, tag="wT")
                    nc.tensor.transpose(wT[:], w[:], one1_sb[:])
                    ee = sp.tile([128, NL], BF16, tag="ee")
                    nc.vector.tensor_tensor(
                        ee[:],
                        mask_sb[:],
                        wT[:].broadcast_to([128, NL]),
                        mybir.AluOpType.mult,
                    )

                    # attention contribution in its own PSUM tile
                    uT = gp.tile([128, GM, NL], FP32, tag="uT")
                    for m in range(GM):
                        nc.tensor.matmul(
                            uT[:, m, :],
                            p_sb[:, m * 128:(m + 1) * 128],
                            ee[:],
                            start=True,
                            stop=True,
                        )

                    # ----- gate math -----
                    spre = sp.tile([128, GM, NL], FP32, tag="spre")
                    nc.vector.tensor_tensor(
                        spre[:].rearrange("p m n -> p (m n)").unsqueeze(2),
                        aT[:].rearrange("p m n -> p (m n)").unsqueeze(2),
                        xwxt[:, :, :, bass.ds(ti, 1)].rearrange("p m n t -> p (m n) t"),
                        mybir.AluOpType.add,
                    )
                    nc.vector.tensor_tensor(
                        spre[:], spre[:], uT[:], mybir.AluOpType.add
                    )
                    gs = sp.tile([128, GM, NL], FP32, tag="gs")
                    fl = spre[:].rearrange("p m n -> p (m n)")
                    gl = gs[:].rearrange("p m n -> p (m n)")
                    q = HC * NL  # 64 columns per gate
                    nc.scalar.activation(
                        gl[:, 0 * q:1 * q], fl[:, 0 * q:1 * q],
                        mybir.ActivationFunctionType.Sigmoid,
                    )
                    nc.scalar.activation(
                        gl[:, 1 * q:2 * q], fl[:, 1 * q:2 * q],
                        mybir.ActivationFunctionType.Sigmoid,
                    )
                    nc.scalar.activation(
                        gl[:, 2 * q:3 * q], fl[:, 2 * q:3 * q],
                        mybir.ActivationFunctionType.Sigmoid,
                    )
                    nc.scalar.activation(
                        gl[:, 3 * q:4 * q], fl[:, 3 * q:4 * q],
                        mybir.ActivationFunctionType.Tanh,
                    )
                    ig = sp.tile([128, HC * NL], FP32, tag="ig")
                    nc.vector.tensor_tensor(
                        ig[:], gl[:, 0 * q:1 * q], gl[:, 3 * q:4 * q],
                        mybir.AluOpType.mult,
                    )
                    cflat = cT[:].rearrange("p c n -> p (c n)")
                    nc.vector.tensor_tensor(
                        cflat, cflat, gl[:, 1 * q:2 * q], mybir.AluOpType.mult
                    )
                    nc.vector.tensor_tensor(cflat, cflat, ig[:], mybir.AluOpType.add)
                    tc_t = sp.tile([128, HC * NL], FP32, tag="tct")
                    nc.scalar.activation(
                        tc_t[:], cflat, mybir.ActivationFunctionType.Tanh
                    )
                    hflat = hT32[:].rearrange("p c n -> p (c n)")
                    nc.vector.tensor_tensor(
                        hflat, gl[:, 2 * q:3 * q], tc_t[:], mybir.AluOpType.mult
                    )
                    nc.vector.tensor_copy(hTb[:], hT32[:])
                    nc.sync.dma_start(
                        out_d[bass.ds(ti, 1), :, :, :].rearrange(
                            "t p c n -> p (t c) n"
                        ),
                        hT32[:],
                    )

    nc.finalize()
    return nc


def prep_inputs(x, A, Wx, Wh, Wattn, b):
    """Host-side reshapes to device layouts; returns per-core input maps."""
    x = np.asarray(x, dtype=np.float32)
    A = np.asarray(A, dtype=np.float32)
    Wx = np.asarray(Wx, dtype=np.float32)
    Wh = np.asarray(Wh, dtype=np.float32)
    Wattn = np.asarray(Wattn, dtype=np.float32)
    b = np.asarray(b, dtype=np.float32)
    timesteps = x.shape[1]

    # weight layouts [p, kc, g] with k = kc*128 + p
    wx_h = np.ascontiguousarray(Wx.reshape(DC, 128, G).transpose(1, 0, 2))
    wh_h = np.ascontiguousarray(
        Wh.reshape(HC, 128, G).transpose(1, 0, 2).astype(ml_dtypes.bfloat16)
    )
    wattn_h = np.ascontiguousarray(Wattn.reshape(HC, 128, G).transpose(1, 0, 2))
    b_h = np.ascontiguousarray(b.reshape(GM, 128).T)  # [p, m]
    mask_h = np.zeros((128, NL), dtype=np.float32)
    for p in range(128):
        mask_h[p, p // L] = 1.0

    in_maps = []
    for c in range(NC):
        xs = x[c * NL:(c + 1) * NL]          # (8, T, 512)
        As = A[c * NL:(c + 1) * NL].reshape(NL, H, L)  # (8, 1024, 16)
        # xT [p, dc, n, t] = x[n, t, dc*128+p]
        xT_h = np.ascontiguousarray(
            xs.reshape(NL, timesteps, DC, 128).transpose(3, 2, 0, 1)
        )
        # afT [p, hc, n, l] = Af[n, hc*128+p, l]
        afT_h = np.ascontiguousarray(
            As.reshape(NL, HC, 128, L).transpose(2, 1, 0, 3)
        )
        in_maps.append(
            {
                "xT": xT_h,
                "afT": afT_h,
                "wx": wx_h,
                "wh": wh_h,
                "wattn": wattn_h,
                "bias": b_h,
                "mask": mask_h,
            }
        )
    return in_maps


_NC_CACHE = {}


def kernel(x, A, Wx, Wh, Wattn, b, trace=False):
    timesteps = x.shape[1]
    key = timesteps
    if key not in _NC_CACHE:
        _NC_CACHE[key] = build_nc(timesteps)
    nc = _NC_CACHE[key]
    in_maps = prep_inputs(x, A, Wx, Wh, Wattn, b)
    res = run_bass_kernel_spmd(nc, in_maps, list(range(NC)), trace=trace)
    outs = []
    for c in range(NC):
        hsT = res.results[c]["hsT"]  # (T, 128, HC, NL)
        # out[n, t, hc*128+p] = hsT[t, p, hc, n]
        outs.append(hsT.transpose(3, 0, 2, 1).reshape(NL, timesteps, H))
    full = np.concatenate(outs, axis=0).astype(np.float32)
    kernel.last_result = res
    return full


# revision 17
# speedup vs baseline: 8.8889x; 1.1811x over previous
"""Trainium2 Bass kernel for the attention-LSTM captioning RNN.

Problem (per full batch): x(64,128,512), A(64,1024,4,4), Wx(512,4096),
Wh(1024,4096), Wattn(1024,4096), b(4096) -> h-sequence (64,128,1024).

Strategy: data-parallel over N across 8 cores (8 samples/core, weights
replicated).  Per core:
  - precompute P[(n,l),g] = Af[n,:,l] @ Wattn  (PE, fp32) -> SBUF bf16
  - precompute xWx^T (gate-major) once (PE, f32r) -> SBUF bf16, indexed
    per step with a strided AP (no per-step DMA)
  - recurrence in transposed ("a^T") layout: gates live on 128 partitions
    (partition = gate-col % 128), batch (8) on the free dim.  Wh is the
    stationary operand (bf16, FWL), h^T the 8-wide moving operand.
    Attention is folded in as a second accumulating matmul with a
    block-diagonal softmax-weight matrix E (128x8) against stationary P.
Host numpy does all layout transposes (free: not timed on device).
"""

import math
import sys

sys.path.insert(0, "/root/shim")
sys.path.insert(0, "/opt/trn_rl_repo")

import numpy as np
import ml_dtypes

try:
    import antenv

    if "/root/shim/antenv" not in list(antenv.__path__):
        antenv.__path__.append("/root/shim/antenv")
except Exception:
    pass

import concourse.bass as bass
import concourse.bacc as bacc
import concourse.mybir as mybir
from concourse.tile import TileContext
from concourse.bass_utils import run_bass_kernel_spmd

FP32 = mybir.dt.float32
F32R = mybir.dt.float32r
BF16 = mybir.dt.bfloat16

# Problem constants (hardcoded per harness contract)
N, T, D, H = 64, 128, 512, 1024
NC = 8            # cores
NL = N // NC      # samples per core = 8
G = 4 * H         # 4096 gate columns
L = 16            # attention locations
HC = H // 128     # 8 h-chunks
GM = G // 128     # 32 gate-col chunks
DC = D // 128     # 4 d-chunks
INV_SQRT_H = 1.0 / math.sqrt(H)


def build_nc(timesteps=T):
    nc = bacc.Bacc()

    # ---- DRAM I/O (host-prepped layouts) ----
    xT_d = nc.dram_tensor("xT", [128, DC, NL, timesteps], FP32, kind="ExternalInput")
    afT_d = nc.dram_tensor("afT", [128, HC, NL, L], FP32, kind="ExternalInput")
    wx_d = nc.dram_tensor("wx", [128, DC, G], FP32, kind="ExternalInput")
    wh_d = nc.dram_tensor("wh", [128, HC, G], BF16, kind="ExternalInput")
    wattn_d = nc.dram_tensor("wattn", [128, HC, G], FP32, kind="ExternalInput")
    b_d = nc.dram_tensor("bias", [128, GM], FP32, kind="ExternalInput")
    mask_d = nc.dram_tensor("mask", [128, NL], FP32, kind="ExternalInput")
    out_d = nc.dram_tensor("hsT", [timesteps, 128, HC, NL], FP32, kind="ExternalOutput")

    with TileContext(nc) as tc:
        # ---------- persistent SBUF ----------
        with tc.tile_pool(name="persist", bufs=1) as pp:
            afTb = pp.tile([128, HC, NL, L], BF16)     # Af^T bf16, (hc,n,l) free
            p_sb = pp.tile([128, G], BF16)             # P[(n,l), g]
            wh_sb = pp.tile([128, HC, G], BF16)        # Wh tiles
            xwxt = pp.tile([128, GM, NL, timesteps], BF16)  # xWx^T (+bias)
            bias_sb = pp.tile([128, GM], FP32)
            mask_sb = pp.tile([128, NL], FP32)
            ones_sb = pp.tile([128, 1], FP32)          # for partition-sum matmul
            one1_sb = pp.tile([1, 1], FP32)            # identity for 1xF transpose
            hT32 = pp.tile([128, HC, NL], FP32)        # h^T fp32 (output + c path)
            hTb = pp.tile([128, HC, NL], BF16)         # h^T bf16 (matmul operand)
            cT = pp.tile([128, HC, NL], FP32)

            nc.sync.dma_start(bias_sb[:], b_d[:])
            nc.sync.dma_start(mask_sb[:], mask_d[:])
            nc.vector.memset(ones_sb[:], 1.0)
            nc.vector.memset(one1_sb[:], 1.0)

            # ---------- P = Af^T @ Wattn  (fp32, one-time) ----------
            with (
                tc.tile_pool(name="wattn", bufs=1) as wap,
                tc.tile_pool(name="wsl", bufs=2) as wslp,
                tc.tile_pool(name="ppsum", bufs=1, space="PSUM") as ppp,
            ):
                afT = wap.tile([128, HC, NL, L], FP32)
                nc.sync.dma_start(afT[:], afT_d[:])
                nc.vector.tensor_copy(afTb[:], afT[:])

                # h0 = mean over l of Af  (reduce innermost l)
                nc.vector.tensor_reduce(
                    hT32[:], afT[:], axis=mybir.AxisListType.X,
                    op=mybir.AluOpType.add,
                )
                nc.vector.tensor_scalar_mul(hT32[:], hT32[:], 1.0 / L)
                nc.vector.tensor_copy(cT[:], hT32[:])
                nc.vector.tensor_copy(hTb[:], hT32[:])

                pps = [ppp.tile([128, 1024], FP32, tag=f"pps{gc}", name=f"pps{gc}") for gc in range(4)]
                for hc in range(HC):
                    wsl = wslp.tile([128, G], FP32, tag="wsl")
                    nc.sync.dma_start(wsl[:], wattn_d[:, hc, :])
                    for gc in range(4):
                        for hf in range(2):
                            nc.tensor.matmul(
                                pps[gc][:, hf * 512:(hf + 1) * 512],
                                afT[:, hc, :, :],
                                wsl[
                                    :,
                                    gc * 1024 + hf * 512:gc * 1024 + (hf + 1) * 512,
                                ],
                                start=(hc == 0),
                                stop=(hc == HC - 1),
                            )
                for gc in range(4):
                    nc.vector.tensor_copy(
                        p_sb[:, gc * 1024:(gc + 1) * 1024], pps[gc][:]
                    )

            # ---------- xWx^T into SBUF bf16 (f32r, one-time) ----------
            with (
                tc.tile_pool(name="xwx", bufs=1) as xp,
                tc.tile_pool(name="xwxs", bufs=2) as xsp,
                tc.tile_pool(name="xwpsum", bufs=1, space="PSUM") as xpp,
            ):
                xT_r = xp.tile([128, DC, NL, timesteps], F32R)
                for dc in range(DC):
                    st2 = xsp.tile([128, NL * timesteps], FP32, tag="stage2")
                    nc.sync.dma_start(
                        st2[:], xT_d[:, dc, :, :].rearrange("p n t -> p (n t)")
                    )
                    nc.vector.tensor_copy(
                        xT_r[:, dc, :, :].rearrange("p n t -> p (n t)"), st2[:]
                    )
                ncols = NL * timesteps  # 1024
                col_chunks = [(s, min(s + 512, ncols)) for s in range(0, ncols, 512)]
                for mg in range(GM // 4):  # groups of 4 gate-chunks
                    xwg = [
                        xpp.tile([128, ncols], FP32, tag=f"xw{i}", name=f"xw{i}")
                        for i in range(4)
                    ]
                    for dc in range(DC):
                        st = xsp.tile([128, 512], FP32, tag="stage")
                        nc.sync.dma_start(
                            st[:], wx_d[:, dc, mg * 512:(mg + 1) * 512]
                        )
                        wxr = xsp.tile([128, 512], F32R, tag="wxr")
                        nc.vector.tensor_copy(wxr[:], st[:])
                        for i in range(4):
                            for (lo, hi) in col_chunks:
                                nc.tensor.matmul(
                                    xwg[i][:, lo:hi],
                                    wxr[:, i * 128:(i + 1) * 128],
                                    xT_r[:, dc, :, :].rearrange(
                                        "p n t -> p (n t)"
                                    )[:, lo:hi],
                                    start=(dc == 0),
                                    stop=(dc == DC - 1),
                                )
                    for i in range(4):
                        m = mg * 4 + i
                        nc.vector.tensor_scalar_add(
                            xwxt[:, m, :, :].rearrange("p n t -> p (n t)"),
                            xwg[i][:],
                            bias_sb[:, m:m + 1],
                        )

            # Wh load (bf16, direct)
            nc.sync.dma_start(wh_sb[:], wh_d[:])

            # ---------- recurrence ----------
            with (
                tc.tile_pool(name="step", bufs=2) as sp,
                tc.tile_pool(name="gpsum", bufs=2, space="PSUM") as gp,
                tc.tile_pool(name="spsum", bufs=2, space="PSUM") as ssp,
            ):
                with tc.For_i(0, timesteps, 1, staggered_reset=True) as ti:
                    aT = gp.tile([128, GM, NL], FP32, tag="aT")
                    # gates = Wh^T-tiles @ h^T   (256 bf16 matmuls)
                    for m in range(GM):
                        for kc in range(HC):
                            nc.tensor.matmul(
                                aT[:, m, :],
                                wh_sb[:, kc, m * 128:(m + 1) * 128],
                                hTb[:, kc, :],
                                start=(kc == 0),
                                stop=(kc == HC - 1),
                            )

                    # ----- attention scores from h (pre-update) -----
                    z = sp.tile([128, NL, L, HC], FP32, tag="z")
                    nc.vector.tensor_tensor(
                        z[:],
                        afTb[:].transpose([0, 2, 3, 1]),   # (p, n, l, hc)
                        hTb[:].transpose([0, 2, 1]).unsqueeze(2).broadcast_to(
                            [128, NL, L, HC]
                        ),
                        mybir.AluOpType.mult,
                    )
                    z2 = sp.tile([128, NL * L], FP32, tag="z2")
                    nc.vector.tensor_reduce(
                        z2[:], z[:], axis=mybir.AxisListType.X, op=mybir.AluOpType.add
                    )
                    sc = ssp.tile([1, NL * L], FP32, tag="sc")
                    nc.tensor.matmul(sc[:], ones_sb[:], z2[:], start=True, stop=True)
                    mx = sp.tile([1, NL], FP32, tag="mx")
                    nc.vector.tensor_reduce(
                        mx[:],
                        sc[:].rearrange("q (n l) -> q n l", n=NL),
                        axis=mybir.AxisListType.X,
                        op=mybir.AluOpType.max,
                    )
                    sd = sp.tile([1, NL * L], FP32, tag="sd")
                    nc.vector.tensor_tensor(
                        sd[:].rearrange("q (n l) -> q n l", n=NL),
                        sc[:].rearrange("q (n l) -> q n l", n=NL),
                        mx[:].unsqueeze(2).broadcast_to([1, NL, L]),
                        mybir.AluOpType.subtract,
                    )
                    ex = sp.tile([1, NL * L], FP32, tag="ex")
                    nc.scalar.activation(
                        ex[:], sd[:], mybir.ActivationFunctionType.Exp,
                        scale=INV_SQRT_H,
                    )
                    zs = sp.tile([1, NL], FP32, tag="zs")
                    nc.vector.tensor_reduce(
                        zs[:],
                        ex[:].rearrange("q (n l) -> q n l", n=NL),
                        axis=mybir.AxisListType.X,
                        op=mybir.AluOpType.add,
                    )
                    rz = sp.tile([1, NL], FP32, tag="rz")
                    nc.vector.reciprocal(rz[:], zs[:])
                    w = sp.tile([1, NL * L], FP32, tag="w")
                    nc.vector.tensor_tensor(
                        w[:].rearrange("q (n l) -> q n l", n=NL),
                        ex[:].rearrange("q (n l) -> q n l", n=NL),
                        rz[:].unsqueeze(2).broadcast_to([1, NL, L]),
                        mybir.AluOpType.mult,
                    )
                    wT = ssp.tile([128, 1], FP32, tag="wT")
                    nc.tensor.transpose(wT[:], w[:], one1_sb[:])
                    ee = sp.tile([128, NL], BF16, tag="ee")
                    nc.vector.tensor_tensor(
                        ee[:],
                        mask_sb[:],
                        wT[:].broadcast_to([128, NL]),
                        mybir.AluOpType.mult,
                    )

                    # attention contribution in its own PSUM tile
                    uT = gp.tile([128, GM, NL], FP32, tag="uT")
                    for m in range(GM):
                        nc.tensor.matmul(
                            uT[:, m, :],
                            p_sb[:, m * 128:(m + 1) * 128],
                            ee[:],
                            start=True,
                            stop=True,
                        )

                    # ----- gate math -----
                    spre = sp.tile([128, GM, NL], FP32, tag="spre")
                    nc.vector.tensor_tensor(
                        spre[:].rearrange("p m n -> p (m n)").unsqueeze(2),
                        aT[:].rearrange("p m n -> p (m n)").unsqueeze(2),
                        xwxt[:, :, :, bass.ds(ti, 1)].rearrange("p m n t -> p (m n) t"),
                        mybir.AluOpType.add,
                    )
                    nc.vector.tensor_tensor(
                        spre[:], spre[:], uT[:], mybir.AluOpType.add
                    )
                    gs = sp.tile([128, GM, NL], FP32, tag="gs")
                    fl = spre[:].rearrange("p m n -> p (m n)")
                    gl = gs[:].rearrange("p m n -> p (m n)")
                    q = HC * NL  # 64 columns per gate
                    nc.scalar.activation(
                        gl[:, 0 * q:1 * q], fl[:, 0 * q:1 * q],
                        mybir.ActivationFunctionType.Sigmoid,
                    )
                    nc.scalar.activation(
                        gl[:, 1 * q:2 * q], fl[:, 1 * q:2 * q],
                        mybir.ActivationFunctionType.Sigmoid,
                    )
                    nc.scalar.activation(
                        gl[:, 2 * q:3 * q], fl[:, 2 * q:3 * q],
                        mybir.ActivationFunctionType.Sigmoid,
                    )
                    nc.scalar.activation(
                        gl[:, 3 * q:4 * q], fl[:, 3 * q:4 * q],
                        mybir.ActivationFunctionType.Tanh,
                    )
                    ig = sp.tile([128, HC * NL], FP32, tag="ig")
                    nc.vector.tensor_tensor(
                        ig[:], gl[:, 0 * q:1 * q], gl[:, 3 * q:4 * q],
                        mybir.AluOpType.mult,
                    )
                    cflat = cT[:].rearrange("p c n -> p (c n)")
                    nc.vector.tensor_tensor(
                        cflat, cflat, gl[:, 1 * q:2 * q], mybir.AluOpType.mult
                    )
                    nc.vector.tensor_tensor(cflat, cflat, ig[:], mybir.AluOpType.add)
                    tc_t = sp.tile([128, HC * NL], FP32, tag="tct")
                    nc.scalar.activation(
                        tc_t[:], cflat, mybir.ActivationFunctionType.Tanh
                    )
                    hflat = hT32[:].rearrange("p c n -> p (c n)")
                    nc.vector.tensor_tensor(
                        hflat, gl[:, 2 * q:3 * q], tc_t[:], mybir.AluOpType.mult
                    )
                    nc.vector.tensor_copy(hTb[:], hT32[:])
                    nc.sync.dma_start(
                        out_d[bass.ds(ti, 1), :, :, :].rearrange(
                            "t p c n -> p (t c) n"
                        ),
                        hT32[:],
                    )

    nc.finalize()
    return nc


def prep_inputs(x, A, Wx, Wh, Wattn, b):
    """Host-side reshapes to device layouts; returns per-core input maps."""
    x = np.asarray(x, dtype=np.float32)
    A = np.asarray(A, dtype=np.float32)
    Wx = np.asarray(Wx, dtype=np.float32)
    Wh = np.asarray(Wh, dtype=np.float32)
    Wattn = np.asarray(Wattn, dtype=np.float32)
    b = np.asarray(b, dtype=np.float32)
    timesteps = x.shape[1]

    # weight layouts [p, kc, g] with k = kc*128 + p
    wx_h = np.ascontiguousarray(Wx.reshape(DC, 128, G).transpose(1, 0, 2))
    wh_h = np.ascontiguousarray(
        Wh.reshape(HC, 128, G).transpose(1, 0, 2).astype(ml_dtypes.bfloat16)
    )
    wattn_h = np.ascontiguousarray(Wattn.reshape(HC, 128, G).transpose(1, 0, 2))
    b_h = np.ascontiguousarray(b.reshape(GM, 128).T)  # [p, m]
    mask_h = np.zeros((128, NL), dtype=np.float32)
    for p in range(128):
        mask_h[p, p // L] = 1.0

    in_maps = []
    for c in range(NC):
        xs = x[c * NL:(c + 1) * NL]          # (8, T, 512)
        As = A[c * NL:(c + 1) * NL].reshape(NL, H, L)  # (8, 1024, 16)
        # xT [p, dc, n, t] = x[n, t, dc*128+p]
        xT_h = np.ascontiguousarray(
            xs.reshape(NL, timesteps, DC, 128).transpose(3, 2, 0, 1)
        )
        # afT [p, hc, n, l] = Af[n, hc*128+p, l]
        afT_h = np.ascontiguousarray(
            As.reshape(NL, HC, 128, L).transpose(2, 1, 0, 3)
        )
        in_maps.append(
            {
                "xT": xT_h,
                "afT": afT_h,
                "wx": wx_h,
                "wh": wh_h,
                "wattn": wattn_h,
                "bias": b_h,
                "mask": mask_h,
            }
        )
    return in_maps


_NC_CACHE = {}


def kernel(x, A, Wx, Wh, Wattn, b, trace=False):
    timesteps = x.shape[1]
    key = timesteps
    if key not in _NC_CACHE:
        _NC_CACHE[key] = build_nc(timesteps)
    nc = _NC_CACHE[key]
    in_maps = prep_inputs(x, A, Wx, Wh, Wattn, b)
    res = run_bass_kernel_spmd(nc, in_maps, list(range(NC)), trace=trace)
    outs = []
    for c in range(NC):
        hsT = res.results[c]["hsT"]  # (T, 128, HC, NL)
        # out[n, t, hc*128+p] = hsT[t, p, hc, n]
        outs.append(hsT.transpose(3, 0, 2, 1).reshape(NL, timesteps, H))
    full = np.concatenate(outs, axis=0).astype(np.float32)
    kernel.last_result = res
    return full


# revision 18
# speedup vs baseline: 9.6131x; 1.0815x over previous
"""Trainium2 Bass kernel for the attention-LSTM captioning RNN.

Problem (per full batch): x(64,128,512), A(64,1024,4,4), Wx(512,4096),
Wh(1024,4096), Wattn(1024,4096), b(4096) -> h-sequence (64,128,1024).

Strategy: data-parallel over N across 8 cores (8 samples/core, weights
replicated).  Per core:
  - precompute P[(n,l),g] = Af[n,:,l] @ Wattn  (PE, fp32) -> SBUF bf16
  - precompute xWx^T (gate-major) once (PE, f32r) -> SBUF bf16, indexed
    per step with a strided AP (no per-step DMA)
  - recurrence in transposed ("a^T") layout: gates live on 128 partitions
    (partition = gate-col % 128), batch (8) on the free dim.  Wh is the
    stationary operand (bf16, FWL), h^T the 8-wide moving operand.
    Attention is folded in as a second accumulating matmul with a
    block-diagonal softmax-weight matrix E (128x8) against stationary P.
Host numpy does all layout transposes (free: not timed on device).
"""

import math
import sys

sys.path.insert(0, "/root/shim")
sys.path.insert(0, "/opt/trn_rl_repo")

import numpy as np
import ml_dtypes

try:
    import antenv

    if "/root/shim/antenv" not in list(antenv.__path__):
        antenv.__path__.append("/root/shim/antenv")
except Exception:
    pass

import concourse.bass as bass
import concourse.bacc as bacc
import concourse.mybir as mybir
from concourse.tile import TileContext
from concourse.bass_utils import run_bass_kernel_spmd

FP32 = mybir.dt.float32
F32R = mybir.dt.float32r
BF16 = mybir.dt.bfloat16

# Problem constants (hardcoded per harness contract)
N, T, D, H = 64, 128, 512, 1024
NC = 8            # cores
NL = N // NC      # samples per core = 8
G = 4 * H         # 4096 gate columns
L = 16            # attention locations
HC = H // 128     # 8 h-chunks
GM = G // 128     # 32 gate-col chunks
DC = D // 128     # 4 d-chunks
INV_SQRT_H = 1.0 / math.sqrt(H)


def build_nc(timesteps=T):
    nc = bacc.Bacc()

    # ---- DRAM I/O (host-prepped layouts) ----
    xT_d = nc.dram_tensor("xT", [128, DC, NL, timesteps], FP32, kind="ExternalInput")
    afT_d = nc.dram_tensor("afT", [128, HC, NL, L], FP32, kind="ExternalInput")
    wx_d = nc.dram_tensor("wx", [128, DC, G], FP32, kind="ExternalInput")
    wh_d = nc.dram_tensor("wh", [128, HC, G], BF16, kind="ExternalInput")
    wattn_d = nc.dram_tensor("wattn", [128, HC, G], FP32, kind="ExternalInput")
    b_d = nc.dram_tensor("bias", [128, GM], FP32, kind="ExternalInput")
    mask_d = nc.dram_tensor("mask", [128, NL], FP32, kind="ExternalInput")
    out_d = nc.dram_tensor("hsT", [timesteps, 128, HC, NL], FP32, kind="ExternalOutput")

    with TileContext(nc) as tc:
        # ---------- persistent SBUF ----------
        with tc.tile_pool(name="persist", bufs=1) as pp:
            afTb = pp.tile([128, HC, NL, L], BF16)     # Af^T bf16, (hc,n,l) free
            p_sb = pp.tile([128, G], BF16)             # P[(n,l), g]
            wh_sb = pp.tile([128, HC, G], BF16)        # Wh tiles
            xwxt = pp.tile([128, GM, NL, timesteps], BF16)  # xWx^T (+bias)
            bias_sb = pp.tile([128, GM], FP32)
            mask_sb = pp.tile([128, NL], FP32)
            ones_sb = pp.tile([128, 1], FP32)          # for partition-sum matmul
            one1_sb = pp.tile([1, 1], FP32)            # identity for 1xF transpose
            hT32 = pp.tile([128, HC, NL], FP32)        # h^T fp32 (output + c path)
            hTb = pp.tile([128, HC, NL], BF16)         # h^T bf16 (matmul operand)
            cT = pp.tile([128, HC, NL], FP32)

            nc.sync.dma_start(bias_sb[:], b_d[:])
            nc.sync.dma_start(mask_sb[:], mask_d[:])
            nc.vector.memset(ones_sb[:], 1.0)
            nc.vector.memset(one1_sb[:], 1.0)

            # ---------- P = Af^T @ Wattn  (fp32, one-time) ----------
            with (
                tc.tile_pool(name="wattn", bufs=1) as wap,
                tc.tile_pool(name="wsl", bufs=2) as wslp,
                tc.tile_pool(name="ppsum", bufs=1, space="PSUM") as ppp,
            ):
                afT = wap.tile([128, HC, NL, L], FP32)
                nc.sync.dma_start(afT[:], afT_d[:])
                nc.vector.tensor_copy(afTb[:], afT[:])

                # h0 = mean over l of Af  (reduce innermost l)
                nc.vector.tensor_reduce(
                    hT32[:], afT[:], axis=mybir.AxisListType.X,
                    op=mybir.AluOpType.add,
                )
                nc.vector.tensor_scalar_mul(hT32[:], hT32[:], 2.0 / L)
                nc.vector.tensor_copy(cT[:], hT32[:])
                nc.vector.tensor_copy(hTb[:], hT32[:])

                pps = [ppp.tile([128, 1024], FP32, tag=f"pps{gc}", name=f"pps{gc}") for gc in range(4)]
                for hc in range(HC):
                    wsl = wslp.tile([128, G], FP32, tag="wsl")
                    nc.sync.dma_start(wsl[:], wattn_d[:, hc, :])
                    for gc in range(4):
                        for hf in range(2):
                            nc.tensor.matmul(
                                pps[gc][:, hf * 512:(hf + 1) * 512],
                                afT[:, hc, :, :],
                                wsl[
                                    :,
                                    gc * 1024 + hf * 512:gc * 1024 + (hf + 1) * 512,
                                ],
                                start=(hc == 0),
                                stop=(hc == HC - 1),
                            )
                for gc in range(4):
                    nc.vector.tensor_copy(
                        p_sb[:, gc * 1024:(gc + 1) * 1024], pps[gc][:]
                    )

            # ---------- xWx^T into SBUF bf16 (f32r, one-time) ----------
            with (
                tc.tile_pool(name="xwx", bufs=1) as xp,
                tc.tile_pool(name="xwxs", bufs=2) as xsp,
                tc.tile_pool(name="xwpsum", bufs=1, space="PSUM") as xpp,
            ):
                xT_r = xp.tile([128, DC, NL, timesteps], F32R)
                for dc in range(DC):
                    st2 = xsp.tile([128, NL * timesteps], FP32, tag="stage2")
                    nc.sync.dma_start(
                        st2[:], xT_d[:, dc, :, :].rearrange("p n t -> p (n t)")
                    )
                    nc.vector.tensor_copy(
                        xT_r[:, dc, :, :].rearrange("p n t -> p (n t)"), st2[:]
                    )
                ncols = NL * timesteps  # 1024
                col_chunks = [(s, min(s + 512, ncols)) for s in range(0, ncols, 512)]
                for mg in range(GM // 4):  # groups of 4 gate-chunks
                    xwg = [
                        xpp.tile([128, ncols], FP32, tag=f"xw{i}", name=f"xw{i}")
                        for i in range(4)
                    ]
                    for dc in range(DC):
                        st = xsp.tile([128, 512], FP32, tag="stage")
                        nc.sync.dma_start(
                            st[:], wx_d[:, dc, mg * 512:(mg + 1) * 512]
                        )
                        wxr = xsp.tile([128, 512], F32R, tag="wxr")
                        nc.vector.tensor_copy(wxr[:], st[:])
                        for i in range(4):
                            for (lo, hi) in col_chunks:
                                nc.tensor.matmul(
                                    xwg[i][:, lo:hi],
                                    wxr[:, i * 128:(i + 1) * 128],
                                    xT_r[:, dc, :, :].rearrange(
                                        "p n t -> p (n t)"
                                    )[:, lo:hi],
                                    start=(dc == 0),
                                    stop=(dc == DC - 1),
                                )
                    for i in range(4):
                        m = mg * 4 + i
                        nc.vector.tensor_scalar_add(
                            xwxt[:, m, :, :].rearrange("p n t -> p (n t)"),
                            xwg[i][:],
                            bias_sb[:, m:m + 1],
                        )

            # Wh load (bf16, direct)
            nc.sync.dma_start(wh_sb[:], wh_d[:])

            # ---------- recurrence ----------
            with (
                tc.tile_pool(name="step", bufs=2) as sp,
                tc.tile_pool(name="gpsum", bufs=2, space="PSUM") as gp,
                tc.tile_pool(name="spsum", bufs=2, space="PSUM") as ssp,
            ):
                with tc.For_i(0, timesteps, 1, staggered_reset=True) as ti:
                    aT = gp.tile([128, GM, NL], FP32, tag="aT")
                    # gates = Wh^T-tiles @ h^T   (256 bf16 matmuls)
                    for m in range(GM):
                        for kc in range(HC):
                            nc.tensor.matmul(
                                aT[:, m, :],
                                wh_sb[:, kc, m * 128:(m + 1) * 128],
                                hTb[:, kc, :],
                                start=(kc == 0),
                                stop=(kc == HC - 1),
                            )

                    # ----- attention scores from h (pre-update) -----
                    z = sp.tile([128, NL, L, HC], FP32, tag="z")
                    nc.vector.tensor_tensor(
                        z[:],
                        afTb[:].transpose([0, 2, 3, 1]),   # (p, n, l, hc)
                        hTb[:].transpose([0, 2, 1]).unsqueeze(2).broadcast_to(
                            [128, NL, L, HC]
                        ),
                        mybir.AluOpType.mult,
                    )
                    z2 = sp.tile([128, NL * L], FP32, tag="z2")
                    nc.vector.tensor_reduce(
                        z2[:], z[:], axis=mybir.AxisListType.X, op=mybir.AluOpType.add
                    )
                    sc = ssp.tile([1, NL * L], FP32, tag="sc")
                    nc.tensor.matmul(sc[:], ones_sb[:], z2[:], start=True, stop=True)
                    ex = sp.tile([1, NL * L], FP32, tag="ex")
                    nc.scalar.activation(
                        ex[:], sc[:], mybir.ActivationFunctionType.Exp,
                        scale=INV_SQRT_H / 2.0,
                    )
                    zs = sp.tile([1, NL], FP32, tag="zs")
                    nc.vector.tensor_reduce(
                        zs[:],
                        ex[:].rearrange("q (n l) -> q n l", n=NL),
                        axis=mybir.AxisListType.X,
                        op=mybir.AluOpType.add,
                    )
                    rz = sp.tile([1, NL], FP32, tag="rz")
                    nc.vector.reciprocal(rz[:], zs[:])
                    w = sp.tile([1, NL * L], FP32, tag="w")
                    nc.vector.tensor_tensor(
                        w[:].rearrange("q (n l) -> q n l", n=NL),
                        ex[:].rearrange("q (n l) -> q n l", n=NL),
                        rz[:].unsqueeze(2).broadcast_to([1, NL, L]),
                        mybir.AluOpType.mult,
                    )
                    wT = ssp.tile([128, 1], FP32, tag="wT")
                    nc.tensor.transpose(wT[:], w[:], one1_sb[:])
                    ee = sp.tile([128, NL], BF16, tag="ee")
                    nc.vector.tensor_tensor(
                        ee[:],
                        mask_sb[:],
                        wT[:].broadcast_to([128, NL]),
                        mybir.AluOpType.mult,
                    )

                    # attention contribution in its own PSUM tile
                    uT = gp.tile([128, GM, NL], FP32, tag="uT")
                    for m in range(GM):
                        nc.tensor.matmul(
                            uT[:, m, :],
                            p_sb[:, m * 128:(m + 1) * 128],
                            ee[:],
                            start=True,
                            stop=True,
                        )

                    # ----- gate math -----
                    spre = sp.tile([128, GM, NL], FP32, tag="spre")
                    nc.vector.tensor_tensor(
                        spre[:].rearrange("p m n -> p (m n)").unsqueeze(2),
                        aT[:].rearrange("p m n -> p (m n)").unsqueeze(2),
                        xwxt[:, :, :, bass.ds(ti, 1)].rearrange("p m n t -> p (m n) t"),
                        mybir.AluOpType.add,
                    )
                    nc.vector.tensor_tensor(
                        spre[:], spre[:], uT[:], mybir.AluOpType.add
                    )
                    gs = sp.tile([128, GM, NL], FP32, tag="gs")
                    fl = spre[:].rearrange("p m n -> p (m n)")
                    gl = gs[:].rearrange("p m n -> p (m n)")
                    q = HC * NL  # 64 columns per gate
                    # i,f,o: tanh(a/2) on pre-halved activations; g: tanh(a)
                    nc.scalar.activation(
                        gl[:, 0:3 * q], fl[:, 0:3 * q],
                        mybir.ActivationFunctionType.Tanh,
                    )
                    nc.scalar.activation(
                        gl[:, 3 * q:4 * q], fl[:, 3 * q:4 * q],
                        mybir.ActivationFunctionType.Tanh, scale=2.0,
                    )
                    # c2' = 0.5*(tf+1)*c2 + (ti+1)*tg   (c2 = 2c)
                    t1 = sp.tile([128, HC * NL], FP32, tag="t1")
                    t2 = sp.tile([128, HC * NL], FP32, tag="t2")
                    cflat = cT[:].rearrange("p c n -> p (c n)")
                    nc.vector.scalar_tensor_tensor(
                        t1[:], gl[:, 1 * q:2 * q], 1.0, cflat,
                        mybir.AluOpType.add, mybir.AluOpType.mult,
                    )
                    nc.vector.scalar_tensor_tensor(
                        t2[:], gl[:, 0 * q:1 * q], 1.0, gl[:, 3 * q:4 * q],
                        mybir.AluOpType.add, mybir.AluOpType.mult,
                    )
                    nc.vector.scalar_tensor_tensor(
                        cflat, t1[:], 0.5, t2[:],
                        mybir.AluOpType.mult, mybir.AluOpType.add,
                    )
                    tc_t = sp.tile([128, HC * NL], FP32, tag="tct")
                    nc.scalar.activation(
                        tc_t[:], cflat, mybir.ActivationFunctionType.Tanh, scale=0.5
                    )
                    # h2 = (to+1)*tanh(c)
                    hflat = hT32[:].rearrange("p c n -> p (c n)")
                    nc.vector.scalar_tensor_tensor(
                        hflat, gl[:, 2 * q:3 * q], 1.0, tc_t[:],
                        mybir.AluOpType.add, mybir.AluOpType.mult,
                    )
                    nc.vector.tensor_copy(hTb[:], hT32[:])
                    nc.sync.dma_start(
                        out_d[bass.ds(ti, 1), :, :, :].rearrange(
                            "t p c n -> p (t c) n"
                        ),
                        hT32[:],
                    )

    nc.finalize()
    return nc


def prep_inputs(x, A, Wx, Wh, Wattn, b):
    """Host-side reshapes to device layouts; returns per-core input maps."""
    x = np.asarray(x, dtype=np.float32)
    A = np.asarray(A, dtype=np.float32)
    Wx = np.asarray(Wx, dtype=np.float32)
    Wh = np.asarray(Wh, dtype=np.float32)
    Wattn = np.asarray(Wattn, dtype=np.float32)
    b = np.asarray(b, dtype=np.float32)
    timesteps = x.shape[1]

    # weight layouts [p, kc, g] with k = kc*128 + p
    wx_h = np.ascontiguousarray(0.5 * Wx.reshape(DC, 128, G).transpose(1, 0, 2))
    wh_h = np.ascontiguousarray(
        (0.25 * Wh.reshape(HC, 128, G).transpose(1, 0, 2)).astype(ml_dtypes.bfloat16)
    )
    wattn_h = np.ascontiguousarray(0.5 * Wattn.reshape(HC, 128, G).transpose(1, 0, 2))
    b_h = np.ascontiguousarray(0.5 * b.reshape(GM, 128).T)  # [p, m]
    mask_h = np.zeros((128, NL), dtype=np.float32)
    for p in range(128):
        mask_h[p, p // L] = 1.0

    in_maps = []
    for c in range(NC):
        xs = x[c * NL:(c + 1) * NL]          # (8, T, 512)
        As = A[c * NL:(c + 1) * NL].reshape(NL, H, L)  # (8, 1024, 16)
        # xT [p, dc, n, t] = x[n, t, dc*128+p]
        xT_h = np.ascontiguousarray(
            xs.reshape(NL, timesteps, DC, 128).transpose(3, 2, 0, 1)
        )
        # afT [p, hc, n, l] = Af[n, hc*128+p, l]
        afT_h = np.ascontiguousarray(
            As.reshape(NL, HC, 128, L).transpose(2, 1, 0, 3)
        )
        in_maps.append(
            {
                "xT": xT_h,
                "afT": afT_h,
                "wx": wx_h,
                "wh": wh_h,
                "wattn": wattn_h,
                "bias": b_h,
                "mask": mask_h,
            }
        )
    return in_maps


_NC_CACHE = {}


def kernel(x, A, Wx, Wh, Wattn, b, trace=False):
    timesteps = x.shape[1]
    key = timesteps
    if key not in _NC_CACHE:
        _NC_CACHE[key] = build_nc(timesteps)
    nc = _NC_CACHE[key]
    in_maps = prep_inputs(x, A, Wx, Wh, Wattn, b)
    res = run_bass_kernel_spmd(nc, in_maps, list(range(NC)), trace=trace)
    outs = []
    for c in range(NC):
        hsT = res.results[c]["hsT"]  # (T, 128, HC, NL)
        # out[n, t, hc*128+p] = hsT[t, p, hc, n]
        outs.append(0.5 * hsT.transpose(3, 0, 2, 1).reshape(NL, timesteps, H))
    full = np.concatenate(outs, axis=0).astype(np.float32)
    kernel.last_result = res
    return full
